# revision 17
# baseline (speedup 1.0000x reference)
"""GraphSAGE encoder (2x SAGEConv + Linear + kriging gather) on 8 trn2 NeuronCores.

Strategy (graph-partition SPMD):
  - Nodes are sharded contiguously across the 8 cores by destination; each core
    owns all in-edges of its nodes. Weights are replicated.
  - Per layer: each core computes xp = relu(X @ Wp + bp) for its shard
    (row-major, bf16), AllGathers xp to a full replicated table in DRAM, then
    gathers the source rows of its edges with dma_gather (512B bf16 rows) and
    segment-sums them with one-hot selector matmuls on the tensor engine
    (S[e, d] = (dstloc[e] == d), built on-chip with an is_equal against iota).
  - Activations are kept feature-major (hT) on chip so weight matrices serve as
    stationary lhsT operands and biases are per-partition activation biases.
  - Host does all index preprocessing (per dst-tile x source-window edge
    bucketing, int16 gather indices, balancing permutation) and the final
    un-permute + map_id gather (pure indexing on the output).
"""

import math

import numpy as np
import ml_dtypes

P = 128
C = 8  # cores
IDX_MAX = 32767  # int16 gather index limit
SG_T = 2  # dst tiles per gather supergroup

BF16 = ml_dtypes.bfloat16


def _ceil_div(a, b):
    return -(-a // b)


# ---------------------------------------------------------------------------
# Host-side preprocessing
# ---------------------------------------------------------------------------


def _balance_tiles(deg_w, T):
    """Greedy assignment of nodes (rows of deg_w [n, W]) into T tiles of P
    slots, balancing the per-window load vectors. Returns pos[n] in [0, T*P)."""
    n, W = deg_w.shape
    order = np.argsort(-deg_w.sum(1), kind="stable")
    loads = np.zeros((T, W), np.int64)
    counts = np.zeros(T, np.int64)
    pos = np.empty(n, np.int64)
    for i in order:
        avail = counts < P
        # score: resulting max-window load if node i joins tile t
        score = (loads + deg_w[i]).max(1)
        score[~avail] = np.iinfo(np.int64).max
        t = int(np.argmin(score))
        pos[i] = t * P + counts[t]
        loads[t] += deg_w[i]
        counts[t] += 1
    return pos


def _prep(inputs):
    x = np.asarray(inputs["x"], np.float32)
    edge_index = np.asarray(inputs["edge_index"], np.int64)
    N, F = x.shape
    HID = np.asarray(inputs["Wl1"]).shape[1]
    REP = np.asarray(inputs["Wl2"]).shape[1]
    MID1 = np.asarray(inputs["Wp1"]).shape[1]
    MID2 = np.asarray(inputs["Wp2"]).shape[1]
    assert F % P == 0 and HID % P == 0 and REP % P == 0

    Ns = _ceil_div(N, C)
    T = _ceil_div(Ns, P)
    NsP = T * P
    cpw = min(C, max(1, IDX_MAX // NsP))  # cores per source window
    n_win = _ceil_div(C, cpw)
    win_rows = [(min((w + 1) * cpw, C) - w * cpw) * NsP for w in range(n_win)]

    src, dst = edge_index[0], edge_index[1]
    s_core = src // Ns
    d_core = dst // Ns
    w_edge = (s_core // cpw).astype(np.int64)

    # per-node, per-window in-degree (drives balancing; same graph both layers)
    deg_w = np.zeros((N, n_win), np.int64)
    np.add.at(deg_w, (dst, w_edge), 1)

    # balancing permutation per core: node -> padded local position
    pos = np.empty(N, np.int64)
    for c in range(C):
        lo, hi = c * Ns, min((c + 1) * Ns, N)
        pos[lo:hi] = _balance_tiles(deg_w[lo:hi], T)

    e_tile = pos[dst] // P
    e_dstloc = pos[dst] % P
    e_widx = (s_core - w_edge * cpw) * NsP + pos[src]  # row within window
    assert e_widx.max() <= IDX_MAX

    # edge counts per (core, tile, window) -> shared chunk schedule
    cnt = np.zeros((C, T, n_win), np.int64)
    np.add.at(cnt, (d_core, e_tile, w_edge), 1)
    n_ch = -(-cnt.max(0) // P)  # [T, n_win]

    # schedule: slot offset of each (t, w) group within its window stream,
    # global chunk ids in (t-major, w, i) order
    win_pos = np.zeros(n_win, np.int64)
    grp_off = np.zeros((T, n_win), np.int64)
    grp_chunk0 = np.zeros((T, n_win), np.int64)
    tile_chunks = [[] for _ in range(T)]  # (global_chunk, w, slot_in_window)
    nch_total = 0
    for t in range(T):
        for w in range(n_win):
            grp_off[t, w] = win_pos[w]
            grp_chunk0[t, w] = nch_total
            for i in range(int(n_ch[t, w])):
                tile_chunks[t].append((nch_total + i, w, int(win_pos[w]) + i * P))
            win_pos[w] += int(n_ch[t, w]) * P
            nch_total += int(n_ch[t, w])
    L_w = [int(win_pos[w]) for w in range(n_win)]
    NCH4 = _ceil_div(max(nch_total, 1), 4) * 4

    # supergroups of SG_T tiles -> per-(sg, w) gather calls
    sgs = [list(range(t0, min(t0 + SG_T, T))) for t0 in range(0, T, SG_T)]
    call_off = np.zeros((len(sgs), n_win), np.int64)
    call_slots = np.zeros((len(sgs), n_win), np.int64)
    for si, tiles in enumerate(sgs):
        for w in range(n_win):
            call_off[si, w] = grp_off[tiles[0], w]
            call_slots[si, w] = sum(int(n_ch[t, w]) * P for t in tiles)

    # ---- per-core tables ------------------------------------------------
    # rank of each edge within its (core, tile, window) group
    grp_key = (d_core * T + e_tile) * n_win + w_edge
    sort_idx = np.argsort(grp_key, kind="stable")
    sorted_key = grp_key[sort_idx]
    grp_start = np.searchsorted(sorted_key, np.arange(C * T * n_win))
    rank_sorted = np.arange(len(src)) - grp_start[sorted_key]
    rank = np.empty(len(src), np.int64)
    rank[sort_idx] = rank_sorted

    idx_streams = np.zeros((C, n_win, max(max(L_w), 16)), np.int16)
    dstloc_tab = np.full((C, P, NCH4), -1.0, BF16)
    e_slot = grp_off[e_tile, w_edge] + rank  # slot within window stream
    e_chunkg = grp_chunk0[e_tile, w_edge] + rank // P
    for c in range(C):
        m = d_core == c
        for w in range(n_win):
            mw = m & (w_edge == w)
            idx_streams[c, w, e_slot[mw]] = e_widx[mw].astype(np.int16)
        dstloc_tab[c, (rank[m] % P), e_chunkg[m]] = e_dstloc[m].astype(np.float32)

    # wrap idx streams: idx j at [j % 16, j // 16], replicated to 128 partitions
    idx_tabs = []
    for w in range(n_win):
        L = max(L_w[w], 16)
        a = idx_streams[:, w, :L].reshape(C, L // 16, 16).transpose(0, 2, 1)
        idx_tabs.append(np.tile(a, (1, C, 1)).copy())  # [C, 128, L/16]

    # per-node reciprocal of in-degree, in (tile, pos) layout
    cnt_n = deg_w.sum(1)
    recip = np.ones((C, P, T), np.float32)
    node_core = np.minimum(np.arange(N) // Ns, C - 1)
    recip[node_core, pos % P, pos // P] = (
        1.0 / np.maximum(cnt_n, 1)).astype(np.float32)

    # xT in permuted order: [C, F, NsP]
    xT = np.zeros((C, F, NsP), BF16)
    for c in range(C):
        lo, hi = c * Ns, min((c + 1) * Ns, N)
        xT[c][:, pos[lo:hi]] = x[lo:hi].T.astype(BF16)

    def b16(name):
        return np.ascontiguousarray(np.asarray(inputs[name], np.float32)).astype(BF16)

    def col128(name, fo):
        v = np.asarray(inputs[name], np.float32)
        return np.ascontiguousarray(v.reshape(fo // P, P).T)  # [P, fo/P] f32

    iota4 = np.tile(np.arange(P, dtype=np.float32).astype(BF16), (P, 4)).reshape(
        P, 4 * P)
    ident = np.eye(P, dtype=np.float32).astype(BF16)
    ones = np.ones((1, P), BF16)

    consts = {
        "w_Wp1": b16("Wp1"), "w_Wl1": b16("Wl1"), "w_Wr1": b16("Wr1"),
        "w_Wp2": b16("Wp2"), "w_Wl2": b16("Wl2"), "w_Wr2": b16("Wr2"),
        "w_Wlin": b16("Wlin"),
        "bp1": b16("bp1").reshape(1, -1), "bp2": b16("bp2").reshape(1, -1),
        "bl1": col128("bl1", HID), "bl2": col128("bl2", REP),
        "blin": col128("blin", REP),
        "iota4": iota4, "ident": ident, "ones": ones,
    }

    cfg = dict(
        N=N, F=F, HID=HID, REP=REP, MID1=MID1, MID2=MID2, Ns=Ns, T=T, NsP=NsP,
        cpw=cpw, n_win=n_win, win_rows=win_rows, L_w=L_w, NCH4=NCH4,
        nch_total=nch_total, sgs=sgs, call_off=call_off, call_slots=call_slots,
        tile_chunks=tile_chunks, pos=pos,
    )

    in_maps = []
    for c in range(C):
        m = dict(consts)
        m["xT"] = np.ascontiguousarray(xT[c])
        m["recip"] = np.ascontiguousarray(recip[c])
        m["dstloc"] = np.ascontiguousarray(dstloc_tab[c])
        for w in range(n_win):
            m[f"idx{w}"] = np.ascontiguousarray(idx_tabs[w][c])
        in_maps.append(m)
    return cfg, in_maps


# ---------------------------------------------------------------------------
# Device program
# ---------------------------------------------------------------------------


def _build(cfg, in_map0):
    import concourse.bass as bass
    import concourse.bacc as bacc
    import concourse.mybir as mybir
    import concourse.tile as tile

    dt = mybir.dt
    AF = mybir.ActivationFunctionType
    ALU = mybir.AluOpType

    F, HID, REP = cfg["F"], cfg["HID"], cfg["REP"]
    T, NsP, n_win = cfg["T"], cfg["NsP"], cfg["n_win"]
    MID1, MID2 = cfg["MID1"], cfg["MID2"]
    sgs, call_off, call_slots = cfg["sgs"], cfg["call_off"], cfg["call_slots"]
    tile_chunks = cfg["tile_chunks"]
    win_rows = cfg["win_rows"]

    nc = bacc.Bacc("TRN2", target_bir_lowering=False, num_devices=C)

    # --- DRAM I/O -------------------------------------------------------
    dram = {}
    for name, arr in in_map0.items():
        np_dt = dt.from_np(arr.dtype)
        dram[name] = nc.dram_tensor(name, list(arr.shape), np_dt,
                                    kind="ExternalInput")
    out_hT = nc.dram_tensor("out_hT", [P, NsP], dt.float32,
                            kind="ExternalOutput")

    from contextlib import ExitStack

    with tile.TileContext(nc, num_cores=C) as tc, ExitStack() as es:
        dpool = es.enter_context(tc.tile_pool(name="dram", bufs=1, space="DRAM"))
        xp_shard = [dpool.tile([NsP, (MID1, MID2)[l]], dt.bfloat16,
                               tag=f"xp{l}_shard", name=f"xp{l}_shard")
                    for l in range(2)]
        xp_full = [dpool.tile([C * NsP, (MID1, MID2)[l]], dt.bfloat16,
                              tag=f"xp{l}_full", name=f"xp{l}_full")
                   for l in range(2)]
        cpool = es.enter_context(tc.tile_pool(name="consts", bufs=1))
        gpools = [es.enter_context(tc.tile_pool(name=f"G{w}", bufs=2))
                  for w in range(n_win)]
        spool = es.enter_context(tc.tile_pool(name="work", bufs=3))
        hpool = es.enter_context(tc.tile_pool(name="acts", bufs=1))
        ppool = es.enter_context(tc.tile_pool(name="ps", bufs=2, space="PSUM"))

        def load_const(name, shape=None):
            arr = in_map0[name]
            t = cpool.tile(list(shape or arr.shape), dt.from_np(arr.dtype),
                           tag=name)
            if shape is None or list(shape) == list(arr.shape):
                nc.sync.dma_start(t[:], dram[name].ap())
            return t

        # weights as [P, kc, F_out] stationary chunks
        def load_w(name):
            arr = in_map0[name]
            kc = arr.shape[0] // P
            t = cpool.tile([P, kc, arr.shape[1]], dt.bfloat16, tag=name)
            for k in range(kc):
                nc.sync.dma_start(t[:, k, :], dram[name].ap()[k * P:(k + 1) * P, :])
            return t

        Wp = [load_w("w_Wp1"), load_w("w_Wp2")]
        Wl = [load_w("w_Wl1"), load_w("w_Wl2")]
        Wr = [load_w("w_Wr1"), load_w("w_Wr2")]
        Wlin = load_w("w_Wlin")
        bp = [load_const("bp1"), load_const("bp2")]
        bl = [load_const("bl1"), load_const("bl2")]
        blin = load_const("blin")
        recip = load_const("recip")
        dstloc = load_const("dstloc")
        ident = load_const("ident")
        ones = load_const("ones")
        iota4 = cpool.tile([P, 4, P], dt.bfloat16, tag="iota4")
        nc.sync.dma_start(iota4[:], dram["iota4"].ap().rearrange(
            "p (a b) -> p a b", a=4))
        idx_sb = []
        for w in range(n_win):
            arr = in_map0[f"idx{w}"]
            t = cpool.tile(list(arr.shape), dt.int16, tag=f"idx{w}")
            nc.sync.dma_start(t[:], dram[f"idx{w}"].ap())
            idx_sb.append(t)

        # resident activations (feature-major)
        XT1 = cpool.tile([P, F // P, NsP], dt.bfloat16, tag="XT1")
        for k in range(F // P):
            nc.sync.dma_start(XT1[:, k, :], dram["xT"].ap()[k * P:(k + 1) * P, :])
        H1T = hpool.tile([P, HID // P, NsP], dt.bfloat16, tag="H1T")
        H2T = hpool.tile([P, REP // P, NsP], dt.bfloat16, tag="H2T")

        import os
        STAGE = int(os.environ.get("GNN_STAGE", "9"))
        # 1: xp only; 2: +allgather; 3: +gather; 4: +selector/mean;
        # 5: +transpose+h (full layer 1); 6: +layer 2; 9: full

        def build_layer(l, XT_sb, kc_in, MID, OUT_sb, kc_out):
            kc_mid = MID // P
            # phase A: xp = relu(X @ Wp + bp), row-major -> xp_shard
            for t in range(T):
                ps = ppool.tile([P, MID], dt.float32, tag="acc")
                for k in range(kc_in):
                    nc.tensor.matmul(out=ps[:], lhsT=XT_sb[:, k, t * P:(t + 1) * P],
                                     rhs=Wp[l][:, k, :], start=(k == 0), stop=False)
                nc.tensor.matmul(out=ps[:], lhsT=ones[:1, :], rhs=bp[l][:1, :],
                                 start=False, stop=True)
                xp_sb = spool.tile([P, MID], dt.bfloat16, tag="xp_sb")
                nc.scalar.activation(xp_sb[:], ps[:], AF.Relu)
                nc.sync.dma_start(xp_shard[l][t * P:(t + 1) * P, :], xp_sb[:])

            if STAGE <= 1:
                return
            # phase B: replicate xp to every core
            nc.gpsimd.collective_compute(
                "AllGather", ALU.bypass,
                replica_groups=[list(range(C))],
                ins=[xp_shard[l].opt()],
                outs=[xp_full[l].opt()],
            )
            if STAGE <= 2:
                return

            # phase C: gather + segment-sum + dense
            row0 = np.cumsum([0] + win_rows)
            for si, tiles in enumerate(sgs):
                Gs = {}
                for w in range(n_win):
                    slots = int(call_slots[si, w])
                    if slots == 0:
                        continue
                    G = gpools[w].tile([P, slots // P, MID], dt.bfloat16,
                                       tag=f"G{w}")
                    off16 = int(call_off[si, w]) // 16
                    nc.gpsimd.dma_gather(
                        out_ap=G[:],
                        in_ap=xp_full[l][row0[w]:row0[w] + win_rows[w], :],
                        idxs_ap=idx_sb[w][:, off16: off16 + slots // 16],
                        num_idxs=slots, num_idxs_reg=slots, elem_size=MID,
                        single_packet=False,
                    )
                    Gs[w] = (G, int(call_off[si, w]))
                if STAGE <= 3:
                    continue
                for t in tiles:
                    chunks = tile_chunks[t]
                    mean_sb = spool.tile([P, MID], dt.bfloat16, tag="mean")
                    if chunks:
                        agg = ppool.tile([P, MID], dt.float32, tag="acc")
                        s4 = {}
                        for j, (gch, w, slot) in enumerate(chunks):
                            g4 = gch // 4
                            if g4 not in s4:
                                S4 = spool.tile([P, 4, P], dt.bfloat16, tag="S4")
                                nc.vector.tensor_tensor(
                                    out=S4[:], in0=iota4[:],
                                    in1=dstloc[:, 4 * g4:4 * g4 + 4]
                                        .to_broadcast([P, 4, P]),
                                    op=ALU.is_equal)
                                s4[g4] = S4
                            G, base = Gs[w]
                            nc.tensor.matmul(
                                out=agg[:], lhsT=s4[g4][:, gch % 4, :],
                                rhs=G[:, (slot - base) // P, :],
                                start=(j == 0), stop=(j == len(chunks) - 1))
                        nc.vector.tensor_scalar(
                            out=mean_sb[:], in0=agg[:],
                            scalar1=recip[:, t:t + 1], scalar2=None,
                            op0=ALU.mult)
                    else:
                        nc.vector.memset(mean_sb[:], 0.0)
                    if STAGE <= 4:
                        continue
                    # transpose mean to feature-major
                    meanT = spool.tile([P, kc_mid, P], dt.bfloat16, tag="meanT")
                    for b in range(kc_mid):
                        tp = ppool.tile([P, P], dt.bfloat16, tag="tp")
                        nc.tensor.transpose(tp[:], mean_sb[:, b * P:(b + 1) * P],
                                            ident[:])
                        nc.vector.tensor_copy(meanT[:, b, :], tp[:])
                    # h_out = relu(Wl.T @ meanT + Wr.T @ XT + bl)
                    for fb in range(kc_out):
                        hp = ppool.tile([P, P], dt.float32, tag="hp")
                        for k in range(kc_mid):
                            nc.tensor.matmul(
                                out=hp[:], lhsT=Wl[l][:, k, fb * P:(fb + 1) * P],
                                rhs=meanT[:, k, :], start=(k == 0), stop=False)
                        for k in range(kc_in):
                            nc.tensor.matmul(
                                out=hp[:], lhsT=Wr[l][:, k, fb * P:(fb + 1) * P],
                                rhs=XT_sb[:, k, t * P:(t + 1) * P],
                                start=False, stop=(k == kc_in - 1))
                        nc.scalar.activation(
                            OUT_sb[:, fb, t * P:(t + 1) * P], hp[:], AF.Relu,
                            bias=bl[l][:, fb:fb + 1])

        build_layer(0, XT1, F // P, MID1, H1T, HID // P)
        if STAGE >= 6:
            build_layer(1, H1T, HID // P, MID2, H2T, REP // P)

        # final linear: hout = h2 @ Wlin + blin (feature-major out)
        for t in range(T):
            fo = spool.tile([P, P], dt.float32, tag="fo")
            if STAGE >= 9:
                hp = ppool.tile([P, P], dt.float32, tag="hp")
                for k in range(REP // P):
                    nc.tensor.matmul(out=hp[:], lhsT=Wlin[:, k, :],
                                     rhs=H2T[:, k, t * P:(t + 1) * P],
                                     start=(k == 0), stop=(k == REP // P - 1))
                nc.scalar.activation(fo[:], hp[:], AF.Identity, bias=blin[:, 0:1])
            else:
                nc.vector.memset(fo[:], 0.0)
            nc.sync.dma_start(out_hT.ap()[:, t * P:(t + 1) * P], fo[:])

    nc.compile()
    return nc


# ---------------------------------------------------------------------------
# Entry point
# ---------------------------------------------------------------------------


def run(inputs, trace=False):
    from concourse.bass_utils import run_bass_kernel_spmd

    cfg, in_maps = _prep(inputs)
    nc = _build(cfg, in_maps[0])
    try:
        res = run_bass_kernel_spmd(nc, in_maps, core_ids=list(range(C)),
                                   trace=trace)
    except ModuleNotFoundError:
        # NTFF profiling hooks unavailable in this container
        res = run_bass_kernel_spmd(nc, in_maps, core_ids=list(range(C)),
                                   trace=False)

    N, REP, Ns = cfg["N"], cfg["REP"], cfg["Ns"]
    pos = cfg["pos"]
    h = np.zeros((N, REP), np.float32)
    for c in range(C):
        lo, hi = c * Ns, min((c + 1) * Ns, N)
        h[lo:hi] = res.results[c]["out_hT"][:, pos[lo:hi]].T
    bs = int(np.asarray(inputs["bs"]))
    map_id = np.asarray(inputs["map_id"], np.int64)
    num_nodes = N // bs
    h3 = h.reshape(bs, num_nodes, REP)
    local_feat = np.take_along_axis(h3, map_id[:, :, None], axis=1).reshape(-1, REP)
    return (local_feat, h), res


def kernel(**inputs):
    return run(inputs, trace=False)[0]
